# revision 18
# baseline (speedup 1.0000x reference)
"""AdaptiveFieldEvolution on 8 TRN2 NeuronCores.

Data-parallel: 16 examples sharded 2-per-core, conv weights replicated.
Per core the whole 50-step evolution runs out of SBUF:
  - state held padded (C=128 partitions, 66x66 free) so the 3x3 SAME conv
    is 9 shifted matmuls accumulating in PSUM (8 chunks of 512 columns).
  - ScalarE applies tanh(+bias) from PSUM and an Abs pass with accum_out
    for the per-partition |delta| sums.
  - VectorE computes dd = tanh - cur and cur' = cur + 0.1*dd.
  - GpSimd accumulates finals += newly * cur' (newly is one-hot over steps).
  - steps / ponder_cost / changes_history are reconstructed on the host
    from the per-(step, example) change scalars the kernel emits.
"""

import os
import sys
from contextlib import ExitStack

import numpy as np

if "/opt/trn_rl_repo" not in sys.path:
    sys.path.insert(0, "/opt/trn_rl_repo")

DT = 0.1
THRESHOLD = 0.01
NSTEPS = 50
B = 16
BPC = 2  # examples per core
NCORES = 8
C = 128
H = W = 64
HP = WP = 66  # padded
FP = HP * WP  # 4356
F = H * W  # 4096
CHUNK_ROWS = 8
NCHUNK = H // CHUNK_ROWS  # 8
CHF = CHUNK_ROWS * W  # 512
# change = sum|new-cur| / (C*H*W); |new-cur| = DT*|dd|
CHANGE_SCALE = float(np.float32(DT) / np.float32(C * H * W))

_NC_CACHE = {}
LAST_RESULTS = None


def build_nc(nsteps=NSTEPS, use_f32r=True, enable_asserts=False, threshold=THRESHOLD):
    import time as _time
    _t0 = _time.time()
    import concourse.bass as bass
    import concourse.tile as tile
    from concourse import bacc, mybir
    from concourse import bass_isa

    f32 = mybir.dt.float32
    mm_dt = mybir.dt.float32r if use_f32r else mybir.dt.float32
    AF = mybir.ActivationFunctionType
    OP = mybir.AluOpType

    nc = bacc.Bacc(
        "TRN2",
        target_bir_lowering=False,
        debug=False,
        enable_asserts=enable_asserts,
    )

    field_d = nc.dram_tensor("field", [BPC, C, H, W], mm_dt, kind="ExternalInput").ap()
    w_d = nc.dram_tensor("W", [C, C, 3, 3], mm_dt, kind="ExternalInput").ap()
    b_d = nc.dram_tensor("b", [C], f32, kind="ExternalInput").ap()
    finals_d = nc.dram_tensor("finals", [BPC, C, H, W], f32, kind="ExternalOutput").ap()
    chist_d = nc.dram_tensor("chist", [nsteps, BPC], f32, kind="ExternalOutput").ap()

    with tile.TileContext(nc) as tc, ExitStack() as ctx:
        persist = ctx.enter_context(tc.tile_pool(name="persist", bufs=1))
        tpool = ctx.enter_context(tc.tile_pool(name="tanh", bufs=4))
        ddpool = ctx.enter_context(tc.tile_pool(name="dd", bufs=6))
        scrpool = ctx.enter_context(tc.tile_pool(name="absscr", bufs=3))
        bigpool = ctx.enter_context(tc.tile_pool(name="big", bufs=1))
        psum_conv = ctx.enter_context(tc.tile_pool(name="pconv", bufs=8, space="PSUM"))

        # --- persistent buffers ---
        # p: height+width padded (C, 66, 66) f32r, conv input only (DMA-fed)
        # c: compact contiguous (C, 64*64) f32r, the evolving state
        p = [[persist.tile([C, FP], mm_dt, name=f"p{e}{q}", tag=f"p{e}{q}")
              for q in range(2)] for e in range(BPC)]
        cst = [persist.tile([C, F], mm_dt, name=f"c{e}", tag=f"c{e}")
               for e in range(BPC)]
        fin = [persist.tile([C, FP], f32, name=f"fin{e}", tag=f"fin{e}")
               for e in range(BPC)]
        wt = [persist.tile([C, C], mm_dt, name=f"wt{j}", tag=f"wt{j}") for j in range(9)]
        bias = persist.tile([C, 1], f32, name="bias", tag="bias")
        colsums = [[persist.tile([C, NCHUNK], f32, name=f"colsums{e}{q}", tag=f"colsums{e}{q}")
                    for q in range(2)] for e in range(BPC)]
        rawsums = [persist.tile([C, BPC], f32, name=f"rawsums{q}", tag=f"rawsums{q}")
                   for q in range(2)]
        raw_all = persist.tile([C, BPC], f32, name="raw_all", tag="raw_all")
        chist_sb = persist.tile([1, nsteps * BPC], f32, name="chist_sb", tag="chist_sb")
        change_t = persist.tile([C, BPC], f32, name="change_t", tag="change_t")
        init_t = persist.tile([C, BPC], f32, name="init_t", tag="init_t")
        thr_t = persist.tile([C, BPC], f32, name="thr_t", tag="thr_t")
        lt_t = persist.tile([C, BPC], f32, name="lt_t", tag="lt_t")
        newly_t = persist.tile([C, BPC], f32, name="newly_t", tag="newly_t")
        notconv = [persist.tile([C, BPC], f32, name=f"notconv{q}", tag=f"notconv{q}")
                   for q in range(2)]
        scale_c = persist.tile([C, BPC], f32, name="scale_c", tag="scale_c")
        eps_c = persist.tile([C, BPC], f32, name="eps_c", tag="eps_c")
        thr_c = persist.tile([C, BPC], f32, name="thr_c", tag="thr_c")

        def p3(ap):  # (C, HP, WP) view of padded buffer
            return ap.rearrange("p (h w) -> p h w", h=HP)

        # --- init ---
        zeros_t = bigpool.tile([C, FP // 4], f32, name="zeros_t", tag="big")
        nc.vector.memset(zeros_t[:], 0.0)
        for e in range(BPC):
            for q in range(2):
                for z in range(4):
                    nc.vector.tensor_copy(
                        p[e][q][:, z * (FP // 4):(z + 1) * (FP // 4)], zeros_t[:])
            nc.vector.memset(fin[e][:], 0.0)
            fe = field_d[e].rearrange("p h w -> p (h w)")
            nc.sync.dma_start(cst[e][:], fe)
            nc.sync.dma_start(p3(p[e][0][:])[:, 1:1 + H, 1:1 + W], field_d[e])
        w_t = w_d.rearrange("o i kh kw -> i o kh kw")
        for j in range(9):
            nc.sync.dma_start(wt[j][:], w_t[:, :, j // 3, j % 3])
        nc.sync.dma_start(bias[:], b_d.rearrange("(p one) -> p one", one=1))
        nc.vector.memset(notconv[0][:], 1.0)
        nc.vector.memset(scale_c[:], CHANGE_SCALE)
        nc.vector.memset(eps_c[:], 1e-8)
        nc.vector.memset(thr_c[:], threshold)

        def emit_stats(j):
            """Stats + finals for step j. Emitted ~one step after j's data is
            produced so every input is stale and no FIFO blocks on it."""
            jq = j % 2
            jn = (j + 1) % 2
            for e in range(BPC):
                nc.vector.tensor_reduce(
                    rawsums[jq][:, e:e + 1], colsums[e][jq][:],
                    axis=mybir.AxisListType.X, op=OP.add)
            nc.gpsimd.partition_all_reduce(
                raw_all[:], rawsums[jq][:], channels=C,
                reduce_op=bass_isa.ReduceOp.add)
            nc.vector.tensor_tensor(change_t[:], raw_all[:], scale_c[:], op=OP.mult)
            nc.vector.tensor_copy(chist_sb[:, j * BPC:(j + 1) * BPC], change_t[:1, :])
            if j == 0:
                nc.vector.tensor_tensor(init_t[:], change_t[:], eps_c[:], op=OP.max)
                nc.vector.tensor_tensor(thr_t[:], init_t[:], thr_c[:], op=OP.mult)
            nc.vector.tensor_tensor(lt_t[:], change_t[:], thr_t[:], op=OP.is_lt)
            nc.vector.tensor_tensor(newly_t[:], lt_t[:], notconv[jq][:], op=OP.mult)
            nc.vector.tensor_tensor(
                notconv[jn][:], notconv[jq][:], newly_t[:], op=OP.subtract)
            # finals accumulation in place (newly is one-hot across steps);
            # reads the padded copy p[jn], untouched until step j+2's DMAs.
            for e in range(BPC):
                nc.vector.scalar_tensor_tensor(
                    fin[e][:], p[e][jn][:].bitcast(f32), newly_t[:, e:e + 1],
                    fin[e][:], op0=OP.mult, op1=OP.add)

        for k in range(nsteps):
            cur_i, nxt_i = k % 2, (k + 1) % 2
            for e in range(BPC):
                pad_cur = p3(p[e][cur_i][:])
                pad_nxt = p3(p[e][nxt_i][:])
                dd_tiles = {}

                def emit_abs(ci):
                    scr = scrpool.tile([C, CHF], f32, name="absscr", tag="absscr")
                    nc.scalar.activation(
                        scr[:], dd_tiles.pop(ci)[:], AF.Abs,
                        accum_out=colsums[e][cur_i][:, ci:ci + 1])

                for c in range(NCHUNK):
                    r0 = c * CHUNK_ROWS
                    ps = psum_conv.tile([C, CHF], f32, name="pconv", tag="pconv")
                    for j in range(9):
                        dy, dx = j // 3, j % 3
                        rhs = pad_cur[:, r0 + dy:r0 + dy + CHUNK_ROWS, dx:dx + W]
                        nc.tensor.matmul(
                            ps[:], wt[j][:], rhs, start=(j == 0), stop=(j == 8))
                    t_c = tpool.tile([C, CHF], f32, name="tanh", tag="tanh")
                    nc.scalar.activation(t_c[:], ps[:], AF.Tanh, bias=bias[:])
                    cch = cst[e][:, c * CHF:(c + 1) * CHF]
                    dd_c = ddpool.tile([C, CHF], f32, name="dd", tag="dd")
                    dd_tiles[c] = dd_c
                    dd_eng = nc.vector if c >= NCHUNK - 2 else nc.gpsimd
                    dd_eng.tensor_tensor(
                        dd_c[:], t_c[:], cch.bitcast(f32), op=OP.subtract)
                    # in-place state update: c += DT * dd  (rounds to f32r)
                    nc.vector.scalar_tensor_tensor(
                        cch, dd_c[:], DT, cch.bitcast(f32), op0=OP.mult, op1=OP.add)
                    # refresh padded conv input for the next step. ex0 via
                    # the sync HWDGE, ex1 via the Pool SWDGE ring: separate
                    # semaphore domains, so next-step ex0 convs never wait on
                    # ex1's (later) refresh.
                    dma_eng = nc.sync if e == 0 else nc.gpsimd
                    dma_eng.dma_start(
                        pad_nxt[:, 1 + r0:1 + r0 + CHUNK_ROWS, 1:1 + W],
                        cch.rearrange("p (r w) -> p r w", w=W))
                    # abs runs 2 chunks late so ScalarE never self-blocks on
                    # the ACT->GpSimd->ACT round trip
                    if c >= 2:
                        emit_abs(c - 2)
                emit_abs(NCHUNK - 2)
                emit_abs(NCHUNK - 1)
            if k >= 1:
                emit_stats(k - 1)

        emit_stats(nsteps - 1)

        # --- epilogue: non-converged examples get the last state ---
        last_i = nsteps % 2
        for e in range(BPC):
            nc.vector.scalar_tensor_tensor(
                fin[e][:], p[e][last_i][:].bitcast(f32), notconv[last_i][:, e:e + 1],
                fin[e][:], op0=OP.mult, op1=OP.add)
            fin3 = fin[e][:].rearrange("p (h w) -> p h w", h=HP)
            nc.sync.dma_start(finals_d[e], fin3[:, 1:1 + H, 1:1 + W])
        nc.sync.dma_start(
            chist_d.rearrange("s b -> (s b)").rearrange("(one f) -> one f", one=1),
            chist_sb[:])

    _t1 = _time.time()
    nc.compile()
    print(f"[build_nc] trace+schedule {_t1 - _t0:.1f}s, bacc compile {_time.time() - _t1:.1f}s", flush=True)
    return nc


def get_nc(nsteps=NSTEPS, use_f32r=True):
    key = (nsteps, use_f32r)
    if key not in _NC_CACHE:
        _NC_CACHE[key] = build_nc(nsteps=nsteps, use_f32r=use_f32r)
    return _NC_CACHE[key]


def host_posteriors(chist, threshold=THRESHOLD):
    """chist: (nsteps, B) f32 change values -> steps, ponder, changes_history.

    Mirrors the device mask logic bit-exactly (f32 compare against
    thr = f32(0.01) * f32(max(change0, 1e-8)))."""
    chist = chist.astype(np.float32)
    nsteps, b = chist.shape
    init = np.maximum(chist[0], np.float32(1e-8))
    thr = (np.float32(threshold) * init).astype(np.float32)
    steps = np.full((b,), np.float32(nsteps), dtype=np.float32)
    conv = np.zeros((b,), dtype=bool)
    for k in range(nsteps):
        newly = (~conv) & (chist[k] < thr)
        steps[newly] = np.float32(k + 1)
        conv |= newly
    ponder = (steps.mean() / np.float32(nsteps)).astype(np.float32)
    changes_history = chist.mean(axis=1).astype(np.float32)
    return steps, ponder, changes_history


def kernel(field, W, b, max_steps):
    from concourse.bass_utils import run_bass_kernel_spmd

    field = np.ascontiguousarray(np.asarray(field, dtype=np.float32))
    Wm = np.ascontiguousarray(np.asarray(W, dtype=np.float32))
    bv = np.ascontiguousarray(np.asarray(b, dtype=np.float32))
    nsteps = int(max_steps)
    assert nsteps == NSTEPS, f"kernel hardcodes {NSTEPS} steps, got {nsteps}"
    assert field.shape == (16, 128, 64, 64)

    use_f32r = os.environ.get("BASSK_F32R", "1") == "1"
    nc = get_nc(nsteps=nsteps, use_f32r=use_f32r)

    in_maps = []
    for i in range(NCORES):
        in_maps.append({
            "field": field[i * BPC:(i + 1) * BPC],
            "W": Wm,
            "b": bv,
        })
    trace = os.environ.get("BASSK_TRACE", "0") == "1"
    tmpdir = None
    if trace:
        import tempfile
        tmpdir = tempfile.mkdtemp(prefix="bassk_trace_", dir="/tmp")
    try:
        res = run_bass_kernel_spmd(
            nc, in_maps, core_ids=list(range(NCORES)), trace=trace, tmpdir=tmpdir)
    except Exception:
        if not trace:
            raise
        res = run_bass_kernel_spmd(nc, in_maps, core_ids=list(range(NCORES)))
    global LAST_RESULTS
    LAST_RESULTS = res
    outs = res.results

    finals = np.concatenate([outs[i]["finals"] for i in range(NCORES)], axis=0)
    chist = np.concatenate([outs[i]["chist"] for i in range(NCORES)], axis=1)
    steps, ponder, changes_history = host_posteriors(chist)
    return finals, steps, ponder, changes_history


# revision 19
# speedup vs baseline: 1.0791x; 1.0791x over previous
"""AdaptiveFieldEvolution on 8 TRN2 NeuronCores.

Data-parallel: 16 examples sharded 2-per-core, conv weights replicated.
Per core the whole 50-step evolution runs out of SBUF:
  - state held padded (C=128 partitions, 66x66 free) so the 3x3 SAME conv
    is 9 shifted matmuls accumulating in PSUM (8 chunks of 512 columns).
  - ScalarE applies tanh(+bias) from PSUM and an Abs pass with accum_out
    for the per-partition |delta| sums.
  - VectorE computes dd = tanh - cur and cur' = cur + 0.1*dd.
  - GpSimd accumulates finals += newly * cur' (newly is one-hot over steps).
  - steps / ponder_cost / changes_history are reconstructed on the host
    from the per-(step, example) change scalars the kernel emits.
"""

import os
import sys
from contextlib import ExitStack

import numpy as np

if "/opt/trn_rl_repo" not in sys.path:
    sys.path.insert(0, "/opt/trn_rl_repo")

DT = 0.1
THRESHOLD = 0.01
NSTEPS = 50
B = 16
BPC = 2  # examples per core
NCORES = 8
C = 128
H = W = 64
HP = WP = 66  # padded
FP = HP * WP  # 4356
F = H * W  # 4096
CHUNK_ROWS = 8
NCHUNK = H // CHUNK_ROWS  # 8
CHF = CHUNK_ROWS * W  # 512
# change = sum|new-cur| / (C*H*W); |new-cur| = DT*|dd|
CHANGE_SCALE = float(np.float32(DT) / np.float32(C * H * W))

_NC_CACHE = {}
LAST_RESULTS = None


def build_nc(nsteps=NSTEPS, use_f32r=True, enable_asserts=False, threshold=THRESHOLD):
    import time as _time
    _t0 = _time.time()
    import concourse.bass as bass
    import concourse.tile as tile
    from concourse import bacc, mybir
    from concourse import bass_isa

    f32 = mybir.dt.float32
    mm_dt = mybir.dt.float32r if use_f32r else mybir.dt.float32
    AF = mybir.ActivationFunctionType
    OP = mybir.AluOpType

    nc = bacc.Bacc(
        "TRN2",
        target_bir_lowering=False,
        debug=False,
        enable_asserts=enable_asserts,
    )

    field_d = nc.dram_tensor("field", [BPC, C, H, W], mm_dt, kind="ExternalInput").ap()
    w_d = nc.dram_tensor("W", [C, C, 3, 3], mm_dt, kind="ExternalInput").ap()
    b_d = nc.dram_tensor("b", [C], f32, kind="ExternalInput").ap()
    finals_d = nc.dram_tensor("finals", [BPC, C, H, W], f32, kind="ExternalOutput").ap()
    chist_d = nc.dram_tensor("chist", [nsteps, BPC], f32, kind="ExternalOutput").ap()

    with tile.TileContext(nc) as tc, ExitStack() as ctx:
        persist = ctx.enter_context(tc.tile_pool(name="persist", bufs=1))
        tpool = ctx.enter_context(tc.tile_pool(name="tanh", bufs=4))
        ddpool = ctx.enter_context(tc.tile_pool(name="dd", bufs=6))
        scrpool = ctx.enter_context(tc.tile_pool(name="absscr", bufs=3))
        bigpool = ctx.enter_context(tc.tile_pool(name="big", bufs=1))
        psum_conv = ctx.enter_context(tc.tile_pool(name="pconv", bufs=8, space="PSUM"))

        # --- persistent buffers ---
        # p: height+width padded (C, 66, 66) f32r, conv input only (DMA-fed)
        # c: compact contiguous (C, 64*64) f32r, the evolving state
        p = [[persist.tile([C, FP], mm_dt, name=f"p{e}{q}", tag=f"p{e}{q}")
              for q in range(2)] for e in range(BPC)]
        cst = [persist.tile([C, F], mm_dt, name=f"c{e}", tag=f"c{e}")
               for e in range(BPC)]
        fin = [persist.tile([C, FP], f32, name=f"fin{e}", tag=f"fin{e}")
               for e in range(BPC)]
        wt = [persist.tile([C, C], mm_dt, name=f"wt{j}", tag=f"wt{j}") for j in range(9)]
        bias = persist.tile([C, 1], f32, name="bias", tag="bias")
        colsums = [[persist.tile([C, NCHUNK], f32, name=f"colsums{e}{q}", tag=f"colsums{e}{q}")
                    for q in range(2)] for e in range(BPC)]
        rawsums = [persist.tile([C, BPC], f32, name=f"rawsums{q}", tag=f"rawsums{q}")
                   for q in range(2)]
        raw_all = persist.tile([C, BPC], f32, name="raw_all", tag="raw_all")
        chist_sb = persist.tile([1, nsteps * BPC], f32, name="chist_sb", tag="chist_sb")
        change_t = persist.tile([C, BPC], f32, name="change_t", tag="change_t")
        init_t = persist.tile([C, BPC], f32, name="init_t", tag="init_t")
        thr_t = persist.tile([C, BPC], f32, name="thr_t", tag="thr_t")
        lt_t = persist.tile([C, BPC], f32, name="lt_t", tag="lt_t")
        newly_t = persist.tile([C, BPC], f32, name="newly_t", tag="newly_t")
        notconv = [persist.tile([C, BPC], f32, name=f"notconv{q}", tag=f"notconv{q}")
                   for q in range(2)]
        scale_c = persist.tile([C, BPC], f32, name="scale_c", tag="scale_c")
        eps_c = persist.tile([C, BPC], f32, name="eps_c", tag="eps_c")
        thr_c = persist.tile([C, BPC], f32, name="thr_c", tag="thr_c")

        def p3(ap):  # (C, HP, WP) view of padded buffer
            return ap.rearrange("p (h w) -> p h w", h=HP)

        # --- init ---
        zeros_t = bigpool.tile([C, FP // 4], f32, name="zeros_t", tag="big")
        nc.vector.memset(zeros_t[:], 0.0)
        for e in range(BPC):
            for q in range(2):
                for z in range(4):
                    nc.vector.tensor_copy(
                        p[e][q][:, z * (FP // 4):(z + 1) * (FP // 4)], zeros_t[:])
            nc.vector.memset(fin[e][:], 0.0)
            fe = field_d[e].rearrange("p h w -> p (h w)")
            nc.sync.dma_start(cst[e][:], fe)
            nc.sync.dma_start(p3(p[e][0][:])[:, 1:1 + H, 1:1 + W], field_d[e])
        w_t = w_d.rearrange("o i kh kw -> i o kh kw")
        for j in range(9):
            nc.sync.dma_start(wt[j][:], w_t[:, :, j // 3, j % 3])
        nc.sync.dma_start(bias[:], b_d.rearrange("(p one) -> p one", one=1))
        nc.vector.memset(notconv[0][:], 1.0)
        nc.vector.memset(scale_c[:], CHANGE_SCALE)
        nc.vector.memset(eps_c[:], 1e-8)
        nc.vector.memset(thr_c[:], threshold)

        def emit_stats(j):
            """Stats + finals for step j. Emitted ~one step after j's data is
            produced so every input is stale and no FIFO blocks on it."""
            jq = j % 2
            jn = (j + 1) % 2
            for e in range(BPC):
                nc.vector.tensor_reduce(
                    rawsums[jq][:, e:e + 1], colsums[e][jq][:],
                    axis=mybir.AxisListType.X, op=OP.add)
            nc.gpsimd.partition_all_reduce(
                raw_all[:], rawsums[jq][:], channels=C,
                reduce_op=bass_isa.ReduceOp.add)
            nc.vector.tensor_tensor(change_t[:], raw_all[:], scale_c[:], op=OP.mult)
            nc.vector.tensor_copy(chist_sb[:, j * BPC:(j + 1) * BPC], change_t[:1, :])
            if j == 0:
                nc.vector.tensor_tensor(init_t[:], change_t[:], eps_c[:], op=OP.max)
                nc.vector.tensor_tensor(thr_t[:], init_t[:], thr_c[:], op=OP.mult)
            nc.vector.tensor_tensor(lt_t[:], change_t[:], thr_t[:], op=OP.is_lt)
            nc.vector.tensor_tensor(newly_t[:], lt_t[:], notconv[jq][:], op=OP.mult)
            nc.vector.tensor_tensor(
                notconv[jn][:], notconv[jq][:], newly_t[:], op=OP.subtract)
            # finals accumulation in place (newly is one-hot across steps);
            # reads the padded copy p[jn], untouched until step j+2's DMAs.
            for e in range(BPC):
                nc.vector.scalar_tensor_tensor(
                    fin[e][:], p[e][jn][:].bitcast(f32), newly_t[:, e:e + 1],
                    fin[e][:], op0=OP.mult, op1=OP.add)

        for k in range(nsteps):
            cur_i, nxt_i = k % 2, (k + 1) % 2
            for e in range(BPC):
                pad_cur = p3(p[e][cur_i][:])
                pad_nxt = p3(p[e][nxt_i][:])
                dd_tiles = {}

                def emit_abs(ci):
                    scr = scrpool.tile([C, CHF], f32, name="absscr", tag="absscr")
                    nc.scalar.activation(
                        scr[:], dd_tiles.pop(ci)[:], AF.Abs,
                        accum_out=colsums[e][cur_i][:, ci:ci + 1])

                for c in range(NCHUNK):
                    r0 = c * CHUNK_ROWS
                    ps = psum_conv.tile([C, CHF], f32, name="pconv", tag="pconv")
                    for j in range(9):
                        dy, dx = j // 3, j % 3
                        rhs = pad_cur[:, r0 + dy:r0 + dy + CHUNK_ROWS, dx:dx + W]
                        nc.tensor.matmul(
                            ps[:], wt[j][:], rhs, start=(j == 0), stop=(j == 8))
                    t_c = tpool.tile([C, CHF], f32, name="tanh", tag="tanh")
                    nc.scalar.activation(t_c[:], ps[:], AF.Tanh, bias=bias[:])
                    cch = cst[e][:, c * CHF:(c + 1) * CHF]
                    dd_c = ddpool.tile([C, CHF], f32, name="dd", tag="dd")
                    dd_tiles[c] = dd_c
                    dd_eng = nc.vector if c >= NCHUNK - 2 else nc.gpsimd
                    dd_eng.tensor_tensor(
                        dd_c[:], t_c[:], cch.bitcast(f32), op=OP.subtract)
                    # in-place state update: c += DT * dd  (rounds to f32r)
                    nc.vector.scalar_tensor_tensor(
                        cch, dd_c[:], DT, cch.bitcast(f32), op0=OP.mult, op1=OP.add)
                    # refresh padded conv input for the next step (HWDGE)
                    nc.sync.dma_start(
                        pad_nxt[:, 1 + r0:1 + r0 + CHUNK_ROWS, 1:1 + W],
                        cch.rearrange("p (r w) -> p r w", w=W))
                    # abs runs 2 chunks late so ScalarE never self-blocks on
                    # the ACT->GpSimd->ACT round trip
                    if c >= 2:
                        emit_abs(c - 2)
                emit_abs(NCHUNK - 2)
                emit_abs(NCHUNK - 1)
            if k >= 1:
                emit_stats(k - 1)

        emit_stats(nsteps - 1)

        # --- epilogue: non-converged examples get the last state ---
        last_i = nsteps % 2
        for e in range(BPC):
            nc.vector.scalar_tensor_tensor(
                fin[e][:], p[e][last_i][:].bitcast(f32), notconv[last_i][:, e:e + 1],
                fin[e][:], op0=OP.mult, op1=OP.add)
            fin3 = fin[e][:].rearrange("p (h w) -> p h w", h=HP)
            nc.sync.dma_start(finals_d[e], fin3[:, 1:1 + H, 1:1 + W])
        nc.sync.dma_start(
            chist_d.rearrange("s b -> (s b)").rearrange("(one f) -> one f", one=1),
            chist_sb[:])

    _t1 = _time.time()
    nc.compile()
    print(f"[build_nc] trace+schedule {_t1 - _t0:.1f}s, bacc compile {_time.time() - _t1:.1f}s", flush=True)
    return nc


def get_nc(nsteps=NSTEPS, use_f32r=True):
    key = (nsteps, use_f32r)
    if key not in _NC_CACHE:
        _NC_CACHE[key] = build_nc(nsteps=nsteps, use_f32r=use_f32r)
    return _NC_CACHE[key]


def host_posteriors(chist, threshold=THRESHOLD):
    """chist: (nsteps, B) f32 change values -> steps, ponder, changes_history.

    Mirrors the device mask logic bit-exactly (f32 compare against
    thr = f32(0.01) * f32(max(change0, 1e-8)))."""
    chist = chist.astype(np.float32)
    nsteps, b = chist.shape
    init = np.maximum(chist[0], np.float32(1e-8))
    thr = (np.float32(threshold) * init).astype(np.float32)
    steps = np.full((b,), np.float32(nsteps), dtype=np.float32)
    conv = np.zeros((b,), dtype=bool)
    for k in range(nsteps):
        newly = (~conv) & (chist[k] < thr)
        steps[newly] = np.float32(k + 1)
        conv |= newly
    ponder = (steps.mean() / np.float32(nsteps)).astype(np.float32)
    changes_history = chist.mean(axis=1).astype(np.float32)
    return steps, ponder, changes_history


def kernel(field, W, b, max_steps):
    from concourse.bass_utils import run_bass_kernel_spmd

    field = np.ascontiguousarray(np.asarray(field, dtype=np.float32))
    Wm = np.ascontiguousarray(np.asarray(W, dtype=np.float32))
    bv = np.ascontiguousarray(np.asarray(b, dtype=np.float32))
    nsteps = int(max_steps)
    assert nsteps == NSTEPS, f"kernel hardcodes {NSTEPS} steps, got {nsteps}"
    assert field.shape == (16, 128, 64, 64)

    use_f32r = os.environ.get("BASSK_F32R", "1") == "1"
    nc = get_nc(nsteps=nsteps, use_f32r=use_f32r)

    in_maps = []
    for i in range(NCORES):
        in_maps.append({
            "field": field[i * BPC:(i + 1) * BPC],
            "W": Wm,
            "b": bv,
        })
    trace = os.environ.get("BASSK_TRACE", "0") == "1"
    tmpdir = None
    if trace:
        import tempfile
        tmpdir = tempfile.mkdtemp(prefix="bassk_trace_", dir="/tmp")
    try:
        res = run_bass_kernel_spmd(
            nc, in_maps, core_ids=list(range(NCORES)), trace=trace, tmpdir=tmpdir)
    except Exception:
        if not trace:
            raise
        res = run_bass_kernel_spmd(nc, in_maps, core_ids=list(range(NCORES)))
    global LAST_RESULTS
    LAST_RESULTS = res
    outs = res.results

    finals = np.concatenate([outs[i]["finals"] for i in range(NCORES)], axis=0)
    chist = np.concatenate([outs[i]["chist"] for i in range(NCORES)], axis=1)
    steps, ponder, changes_history = host_posteriors(chist)
    return finals, steps, ponder, changes_history


# revision 21
# speedup vs baseline: 1.5520x; 1.4382x over previous
"""AdaptiveFieldEvolution on 8 TRN2 NeuronCores.

Data-parallel: 16 examples sharded 2-per-core, conv weights replicated.
Per core the whole 50-step evolution runs out of SBUF:
  - state held padded (C=128 partitions, 66x66 free) so the 3x3 SAME conv
    is 9 shifted matmuls accumulating in PSUM (8 chunks of 512 columns).
  - ScalarE applies tanh(+bias) from PSUM and an Abs pass with accum_out
    for the per-partition |delta| sums.
  - VectorE computes dd = tanh - cur and cur' = cur + 0.1*dd.
  - GpSimd accumulates finals += newly * cur' (newly is one-hot over steps).
  - steps / ponder_cost / changes_history are reconstructed on the host
    from the per-(step, example) change scalars the kernel emits.
"""

import os
import sys
from contextlib import ExitStack

import numpy as np

if "/opt/trn_rl_repo" not in sys.path:
    sys.path.insert(0, "/opt/trn_rl_repo")

DT = 0.1
THRESHOLD = 0.01
NSTEPS = 50
B = 16
BPC = 2  # examples per core
NCORES = 8
C = 128
H = W = 64
HP = WP = 66  # padded
FP = HP * WP  # 4356
F = H * W  # 4096
CHUNK_ROWS = 8
NCHUNK = H // CHUNK_ROWS  # 8
CHF = CHUNK_ROWS * W  # 512
# change = sum|new-cur| / (C*H*W); |new-cur| = DT*|dd|
CHANGE_SCALE = float(np.float32(DT) / np.float32(C * H * W))

_NC_CACHE = {}
LAST_RESULTS = None


def build_nc(nsteps=NSTEPS, use_f32r=True, enable_asserts=False, threshold=THRESHOLD):
    import time as _time
    _t0 = _time.time()
    import concourse.bass as bass
    import concourse.tile as tile
    from concourse import bacc, mybir
    from concourse import bass_isa

    f32 = mybir.dt.float32
    mm_dt = mybir.dt.float32r if use_f32r else mybir.dt.float32
    AF = mybir.ActivationFunctionType
    OP = mybir.AluOpType

    nc = bacc.Bacc(
        "TRN2",
        target_bir_lowering=False,
        debug=False,
        enable_asserts=enable_asserts,
    )

    field_d = nc.dram_tensor("field", [BPC, C, H, W], mm_dt, kind="ExternalInput").ap()
    w_d = nc.dram_tensor("W", [C, C, 3, 3], mm_dt, kind="ExternalInput").ap()
    b_d = nc.dram_tensor("b", [C], f32, kind="ExternalInput").ap()
    finals_d = nc.dram_tensor("finals", [BPC, C, H, W], f32, kind="ExternalOutput").ap()
    chist_d = nc.dram_tensor("chist", [nsteps, BPC], f32, kind="ExternalOutput").ap()

    with tile.TileContext(nc) as tc, ExitStack() as ctx:
        persist = ctx.enter_context(tc.tile_pool(name="persist", bufs=1))
        tpool = ctx.enter_context(tc.tile_pool(name="tanh", bufs=4))
        ddpool = ctx.enter_context(tc.tile_pool(name="dd", bufs=6))
        scrpool = ctx.enter_context(tc.tile_pool(name="absscr", bufs=3))
        bigpool = ctx.enter_context(tc.tile_pool(name="big", bufs=1))
        psum_conv = ctx.enter_context(tc.tile_pool(name="pconv", bufs=8, space="PSUM"))

        # --- persistent buffers ---
        # p: height+width padded (C, 66, 66) f32r, conv input only (DMA-fed)
        # c: compact contiguous (C, 64*64) f32r, the evolving state
        p = [[persist.tile([C, FP], mm_dt, name=f"p{e}{q}", tag=f"p{e}{q}")
              for q in range(2)] for e in range(BPC)]
        cst = [persist.tile([C, F], mm_dt, name=f"c{e}", tag=f"c{e}")
               for e in range(BPC)]
        fin = [persist.tile([C, FP], f32, name=f"fin{e}", tag=f"fin{e}")
               for e in range(BPC)]
        wt = [persist.tile([C, C], mm_dt, name=f"wt{j}", tag=f"wt{j}") for j in range(9)]
        bias = persist.tile([C, 1], f32, name="bias", tag="bias")
        colsums = [[persist.tile([C, NCHUNK], f32, name=f"colsums{e}{q}", tag=f"colsums{e}{q}")
                    for q in range(2)] for e in range(BPC)]
        rawsums = [persist.tile([C, BPC], f32, name=f"rawsums{q}", tag=f"rawsums{q}")
                   for q in range(2)]
        raw_all = persist.tile([C, BPC], f32, name="raw_all", tag="raw_all")
        chist_sb = persist.tile([1, nsteps * BPC], f32, name="chist_sb", tag="chist_sb")
        change_t = persist.tile([C, BPC], f32, name="change_t", tag="change_t")
        init_t = persist.tile([C, BPC], f32, name="init_t", tag="init_t")
        thr_t = persist.tile([C, BPC], f32, name="thr_t", tag="thr_t")
        lt_t = persist.tile([C, BPC], f32, name="lt_t", tag="lt_t")
        newly_t = persist.tile([C, BPC], f32, name="newly_t", tag="newly_t")
        notconv = [persist.tile([C, BPC], f32, name=f"notconv{q}", tag=f"notconv{q}")
                   for q in range(2)]
        scale_c = persist.tile([C, BPC], f32, name="scale_c", tag="scale_c")
        eps_c = persist.tile([C, BPC], f32, name="eps_c", tag="eps_c")
        thr_c = persist.tile([C, BPC], f32, name="thr_c", tag="thr_c")

        def p3(ap):  # (C, HP, WP) view of padded buffer
            return ap.rearrange("p (h w) -> p h w", h=HP)

        # --- init ---
        zeros_t = bigpool.tile([C, FP // 4], f32, name="zeros_t", tag="big")
        nc.vector.memset(zeros_t[:], 0.0)
        for e in range(BPC):
            for q in range(2):
                for z in range(4):
                    nc.vector.tensor_copy(
                        p[e][q][:, z * (FP // 4):(z + 1) * (FP // 4)], zeros_t[:])
            nc.vector.memset(fin[e][:], 0.0)
            fe = field_d[e].rearrange("p h w -> p (h w)")
            nc.sync.dma_start(cst[e][:], fe)
            nc.sync.dma_start(p3(p[e][0][:])[:, 1:1 + H, 1:1 + W], field_d[e])
        w_t = w_d.rearrange("o i kh kw -> i o kh kw")
        for j in range(9):
            nc.sync.dma_start(wt[j][:], w_t[:, :, j // 3, j % 3])
        nc.sync.dma_start(bias[:], b_d.rearrange("(p one) -> p one", one=1))
        nc.vector.memset(notconv[0][:], 1.0)
        nc.vector.memset(scale_c[:], CHANGE_SCALE)
        nc.vector.memset(eps_c[:], 1e-8)
        nc.vector.memset(thr_c[:], threshold)

        import os as _os
        _nostats = _os.environ.get("BASSK_NOSTATS", "0") == "1"
        _noabs = _os.environ.get("BASSK_NOABS", "0") == "1"
        _nofin = _os.environ.get("BASSK_NOFIN", "0") == "1"

        def emit_stats(j):
            if _nostats:
                return
            """Stats + finals for step j. Emitted ~one step after j's data is
            produced so every input is stale and no FIFO blocks on it."""
            jq = j % 2
            jn = (j + 1) % 2
            for e in range(BPC):
                nc.vector.tensor_reduce(
                    rawsums[jq][:, e:e + 1], colsums[e][jq][:],
                    axis=mybir.AxisListType.X, op=OP.add)
            nc.gpsimd.partition_all_reduce(
                raw_all[:], rawsums[jq][:], channels=C,
                reduce_op=bass_isa.ReduceOp.add)
            nc.vector.tensor_tensor(change_t[:], raw_all[:], scale_c[:], op=OP.mult)
            nc.vector.tensor_copy(chist_sb[:, j * BPC:(j + 1) * BPC], change_t[:1, :])
            if j == 0:
                nc.vector.tensor_tensor(init_t[:], change_t[:], eps_c[:], op=OP.max)
                nc.vector.tensor_tensor(thr_t[:], init_t[:], thr_c[:], op=OP.mult)
            nc.vector.tensor_tensor(lt_t[:], change_t[:], thr_t[:], op=OP.is_lt)
            nc.vector.tensor_tensor(newly_t[:], lt_t[:], notconv[jq][:], op=OP.mult)
            nc.vector.tensor_tensor(
                notconv[jn][:], notconv[jq][:], newly_t[:], op=OP.subtract)
            # finals accumulation in place (newly is one-hot across steps);
            # reads the padded copy p[jn], untouched until step j+2's DMAs.
            if not _nofin:
                for e in range(BPC):
                    nc.vector.scalar_tensor_tensor(
                        fin[e][:], p[e][jn][:].bitcast(f32), newly_t[:, e:e + 1],
                        fin[e][:], op0=OP.mult, op1=OP.add)

        for k in range(nsteps):
            cur_i, nxt_i = k % 2, (k + 1) % 2
            for e in range(BPC):
                pad_cur = p3(p[e][cur_i][:])
                pad_nxt = p3(p[e][nxt_i][:])
                dd_tiles = {}

                def emit_abs(ci):
                    if _noabs:
                        dd_tiles.pop(ci)
                        return
                    scr = scrpool.tile([C, CHF], f32, name="absscr", tag="absscr")
                    nc.scalar.activation(
                        scr[:], dd_tiles.pop(ci)[:], AF.Abs,
                        accum_out=colsums[e][cur_i][:, ci:ci + 1])

                for c in range(NCHUNK):
                    r0 = c * CHUNK_ROWS
                    ps = psum_conv.tile([C, CHF], f32, name="pconv", tag="pconv")
                    for j in range(9):
                        dy, dx = j // 3, j % 3
                        rhs = pad_cur[:, r0 + dy:r0 + dy + CHUNK_ROWS, dx:dx + W]
                        nc.tensor.matmul(
                            ps[:], wt[j][:], rhs, start=(j == 0), stop=(j == 8))
                    t_c = tpool.tile([C, CHF], f32, name="tanh", tag="tanh")
                    nc.scalar.activation(t_c[:], ps[:], AF.Tanh, bias=bias[:])
                    cch = cst[e][:, c * CHF:(c + 1) * CHF]
                    dd_c = ddpool.tile([C, CHF], f32, name="dd", tag="dd")
                    dd_tiles[c] = dd_c
                    dd_eng = nc.vector if c >= NCHUNK - 2 else nc.gpsimd
                    dd_eng.tensor_tensor(
                        dd_c[:], t_c[:], cch.bitcast(f32), op=OP.subtract)
                    # in-place state update: c += DT * dd  (rounds to f32r)
                    nc.vector.scalar_tensor_tensor(
                        cch, dd_c[:], DT, cch.bitcast(f32), op0=OP.mult, op1=OP.add)
                    # refresh padded conv input for the next step (HWDGE)
                    nc.sync.dma_start(
                        pad_nxt[:, 1 + r0:1 + r0 + CHUNK_ROWS, 1:1 + W],
                        cch.rearrange("p (r w) -> p r w", w=W))
                    # abs runs 2 chunks late so ScalarE never self-blocks on
                    # the ACT->GpSimd->ACT round trip
                    if c >= 2:
                        emit_abs(c - 2)
                emit_abs(NCHUNK - 2)
                emit_abs(NCHUNK - 1)
            if k >= 1:
                emit_stats(k - 1)

        emit_stats(nsteps - 1)

        # --- epilogue: non-converged examples get the last state ---
        last_i = nsteps % 2
        for e in range(BPC):
            nc.vector.scalar_tensor_tensor(
                fin[e][:], p[e][last_i][:].bitcast(f32), notconv[last_i][:, e:e + 1],
                fin[e][:], op0=OP.mult, op1=OP.add)
            fin3 = fin[e][:].rearrange("p (h w) -> p h w", h=HP)
            nc.sync.dma_start(finals_d[e], fin3[:, 1:1 + H, 1:1 + W])
        if _nostats:
            nc.vector.memset(chist_sb[:], 0.0)
        nc.sync.dma_start(
            chist_d.rearrange("s b -> (s b)").rearrange("(one f) -> one f", one=1),
            chist_sb[:])

    _t1 = _time.time()
    nc.compile()
    print(f"[build_nc] trace+schedule {_t1 - _t0:.1f}s, bacc compile {_time.time() - _t1:.1f}s", flush=True)
    return nc


def get_nc(nsteps=NSTEPS, use_f32r=True):
    key = (nsteps, use_f32r)
    if key not in _NC_CACHE:
        _NC_CACHE[key] = build_nc(nsteps=nsteps, use_f32r=use_f32r)
    return _NC_CACHE[key]


def host_posteriors(chist, threshold=THRESHOLD):
    """chist: (nsteps, B) f32 change values -> steps, ponder, changes_history.

    Mirrors the device mask logic bit-exactly (f32 compare against
    thr = f32(0.01) * f32(max(change0, 1e-8)))."""
    chist = chist.astype(np.float32)
    nsteps, b = chist.shape
    init = np.maximum(chist[0], np.float32(1e-8))
    thr = (np.float32(threshold) * init).astype(np.float32)
    steps = np.full((b,), np.float32(nsteps), dtype=np.float32)
    conv = np.zeros((b,), dtype=bool)
    for k in range(nsteps):
        newly = (~conv) & (chist[k] < thr)
        steps[newly] = np.float32(k + 1)
        conv |= newly
    ponder = (steps.mean() / np.float32(nsteps)).astype(np.float32)
    changes_history = chist.mean(axis=1).astype(np.float32)
    return steps, ponder, changes_history


def kernel(field, W, b, max_steps):
    from concourse.bass_utils import run_bass_kernel_spmd

    field = np.ascontiguousarray(np.asarray(field, dtype=np.float32))
    Wm = np.ascontiguousarray(np.asarray(W, dtype=np.float32))
    bv = np.ascontiguousarray(np.asarray(b, dtype=np.float32))
    nsteps = int(max_steps)
    assert nsteps == NSTEPS, f"kernel hardcodes {NSTEPS} steps, got {nsteps}"
    assert field.shape == (16, 128, 64, 64)

    use_f32r = os.environ.get("BASSK_F32R", "1") == "1"
    nc = get_nc(nsteps=nsteps, use_f32r=use_f32r)

    in_maps = []
    for i in range(NCORES):
        in_maps.append({
            "field": field[i * BPC:(i + 1) * BPC],
            "W": Wm,
            "b": bv,
        })
    trace = os.environ.get("BASSK_TRACE", "0") == "1"
    tmpdir = None
    if trace:
        import tempfile
        tmpdir = tempfile.mkdtemp(prefix="bassk_trace_", dir="/tmp")
    try:
        res = run_bass_kernel_spmd(
            nc, in_maps, core_ids=list(range(NCORES)), trace=trace, tmpdir=tmpdir)
    except Exception:
        if not trace:
            raise
        res = run_bass_kernel_spmd(nc, in_maps, core_ids=list(range(NCORES)))
    global LAST_RESULTS
    LAST_RESULTS = res
    outs = res.results

    finals = np.concatenate([outs[i]["finals"] for i in range(NCORES)], axis=0)
    chist = np.concatenate([outs[i]["chist"] for i in range(NCORES)], axis=1)
    steps, ponder, changes_history = host_posteriors(chist)
    return finals, steps, ponder, changes_history
